# revision 2
# baseline (speedup 1.0000x reference)
"""Trainium2 Bass kernel for a 2-layer LSTM binary classifier.

Block-parallel Picard iteration: instead of a latency-bound serial
recurrence (~1.8us/step chain), process T in blocks of L=32 steps.
Within a block, iterate K times:
  gates = xp + Whh @ h_field        (h_field = stale estimates, bf16)
  sig   = sigmoid(gates)            (one big ACT op)
  u'    = (sig_g2 - 0.5) * sig_i    (c~ = c/2 space)
  c~    = scan: c~ = sig_f * c~ + u'   (exact, tensor_tensor_scan)
  tanh_c = tanh(2*c~)               (ACT, scale=2)
  h_new = sig_o * tanh_c            (bf16)
Per iteration the gates PSUM is updated with Whh @ (h_new - h_old), so
the matmuls stay small and the PSUM accumulates the converged value.
Convergence factor ~0.1/iter (measured): K0=5 (layer0), K1=4 (layer1)
gives final rel err ~2.5e-3 (incl bf16 h-fields), vs gate 2e-2.

The per-example scan boundary is handled with a gap slot: fields have
L+1 slots per example; slot 0 carries (f=0, u=c~_init) so one
tensor_tensor_scan over the whole [128, 8*(L+1)] field resets correctly
at each example boundary and performs the cross-block state handoff.

Sharding: data-parallel over batch (64 -> 8 cores x 8 examples), all
weights replicated.  The xp GEMM (f32r; block 0 in bf16 to shorten the
DMA prologue) writes directly into each block's gates PSUM tile and is
emitted incrementally between iteration ops.  Layer 1 (lagging one
block) interleaves into layer 0's iteration slots: its K1=4 iterations
fill layer 0's K0=5 slots of the next block, so engines overlap.
"""

import numpy as np
import ml_dtypes

import concourse.bass as bass
import concourse.tile as tile
from concourse import bacc, mybir
from concourse.bass_utils import run_bass_kernel_spmd

F32 = mybir.dt.float32
F32R = mybir.dt.float32r
BF16 = mybir.dt.bfloat16
AF = mybir.ActivationFunctionType
OP = mybir.AluOpType

H = 128          # hidden
D = 2048         # input size
B = 64           # batch
T = 256          # seq len
NCORES = 8
BS = B // NCORES          # 8 examples per core
KT = D // 128             # 16 k-tiles of the input GEMM
L = 32                    # picard block length (timesteps)
NB = T // L               # 8 blocks
LP = L + 1                # field slots per example (slot 0 = gap/init)
K0 = 5                    # picard iterations, layer 0
K1 = 4                    # picard iterations, layer 1
NTOK = BS * L             # 256 tokens per block


def build_program():
    nc = bacc.Bacc("TRN2", target_bir_lowering=False, debug=False,
                   enable_asserts=False)

    # ---- DRAM I/O ----
    xsd = nc.dram_tensor("xs", [NB, 128, KT, BS, L], F32R,
                         kind="ExternalInput").ap()
    x0bfd = nc.dram_tensor("x0bf", [128, KT, BS, L], BF16,
                           kind="ExternalInput").ap()
    wih0d = nc.dram_tensor("wih0t", [128, KT, 4, 128], F32R,
                           kind="ExternalInput").ap()
    wih0bfd = nc.dram_tensor("wih0bf", [128, KT, 4, 128], BF16,
                             kind="ExternalInput").ap()
    whh0d = nc.dram_tensor("whh0t", [128, 4, 128], BF16,
                           kind="ExternalInput").ap()
    whh1d = nc.dram_tensor("whh1t", [128, 4, 128], BF16,
                           kind="ExternalInput").ap()
    wih1d = nc.dram_tensor("wih1t", [128, 4, 128], BF16,
                           kind="ExternalInput").ap()
    b0d = nc.dram_tensor("b0s", [1, 4, 128], BF16, kind="ExternalInput").ap()
    b1d = nc.dram_tensor("b1s", [1, 4, 128], BF16, kind="ExternalInput").ap()
    onesd = nc.dram_tensor("ones256", [1, NTOK], BF16,
                           kind="ExternalInput").ap()
    wfcd = nc.dram_tensor("wfct", [128, 1], BF16, kind="ExternalInput").ap()
    bfcd = nc.dram_tensor("bfcb", [BS, 1], F32, kind="ExternalInput").ap()
    yd = nc.dram_tensor("y", [BS, 1], F32, kind="ExternalOutput").ap()

    with tile.TileContext(nc) as tc, \
            tc.tile_pool(name="persist", bufs=1) as pp:
        # ---- persistent SBUF ----
        wih0t_s = pp.tile([128, KT, 4, 128], F32R, name="wih0t_s")
        wih0bf_s = pp.tile([128, KT, 4, 128], BF16, name="wih0bf_s")
        x0bf_s = pp.tile([128, KT, BS, L], BF16, name="x0bf_s")
        whh0t_s = pp.tile([128, 4, 128], BF16, name="whh0t_s")
        whh1t_s = pp.tile([128, 4, 128], BF16, name="whh1t_s")
        wih1t_s = pp.tile([128, 4, 128], BF16, name="wih1t_s")
        b0s_s = pp.tile([1, 4, 128], BF16, name="b0s_s")
        b1s_s = pp.tile([1, 4, 128], BF16, name="b1s_s")
        ones_s = pp.tile([1, NTOK], BF16, name="ones_s")
        wfct_s = pp.tile([128, 1], BF16, name="wfct_s")
        bfcb_s = pp.tile([BS, 1], F32, name="bfcb_s")
        y_sb = pp.tile([BS, 1], F32, name="y_sb")

        # per-layer picard fields
        SIG = [pp.tile([128, 4, BS, LP], F32, name=f"SIG{l}") for l in range(2)]
        U = [pp.tile([128, BS, LP], F32, name=f"U{l}") for l in range(2)]
        CT = [pp.tile([128, BS, LP], F32, name=f"CT{l}") for l in range(2)]
        TC = [pp.tile([128, BS, L], F32, name=f"TC{l}") for l in range(2)]
        HF = [pp.tile([128, 2, BS, LP], BF16, name=f"HF{l}") for l in range(2)]
        DH = [pp.tile([128, BS, L], BF16, name=f"DH{l}") for l in range(2)]
        H0FIN = pp.tile([128, BS, L], BF16, name="H0FIN")

        # GEMM bias consts must precede block-0 x DMAs on the sync queue
        nc.sync.dma_start(b0s_s[:], b0d[:])
        nc.sync.dma_start(ones_s[:], onesd[:])
        for _k2 in range(0, KT, 2):
            nc.gpsimd.dma_start(wih0bf_s[:, _k2:_k2 + 2],
                                wih0bfd[:, _k2:_k2 + 2])
        for _k in range(KT):
            nc.gpsimd.dma_start(wih0t_s[:, _k], wih0d[:, _k])

        # one-time zero init: gap slots (f-gate slot0 must be exactly 0 so
        # the scan resets at example boundaries), block-0 state
        for l in range(2):
            nc.vector.memset(SIG[l][:, :, :, 0:1], 0.0)
            nc.vector.memset(U[l][:, :, 0:1], 0.0)
            nc.vector.memset(HF[l][:], 0.0)

        def _late_const_dmas():
            nc.sync.dma_start(whh0t_s[:], whh0d[:])
            nc.sync.dma_start(whh1t_s[:], whh1d[:])
            nc.sync.dma_start(wih1t_s[:], wih1d[:])
            nc.sync.dma_start(b1s_s[:], b1d[:])
            nc.sync.dma_start(wfct_s[:], wfcd[:])
            nc.sync.dma_start(bfcb_s[:], bfcd[:])

        with (
            tc.tile_pool(name="xchunk", bufs=2) as x_pool,
            tc.tile_pool(name="g0ps", bufs=2, space="PSUM") as g0_pool,
            tc.tile_pool(name="g1ps", bufs=1, space="PSUM") as g1_pool,
        ):
            g0_tiles = {}

            def gemm_gen(bl):
                """Emit block bl's xp GEMM directly into its gates PSUM."""
                if bl == 0:
                    xt, wmat = x0bf_s, wih0bf_s
                else:
                    xt = x_pool.tile([128, KT, BS, L], F32R, name="xt")
                    wmat = wih0t_s
                for k in range(KT):
                    if bl == 0:
                        nc.sync.dma_start(xt[:, k], x0bfd[:, k])
                    else:
                        nc.sync.dma_start(xt[:, k], xsd[bl, :, k])
                    yield 1
                P = g0_pool.tile([128, 4, BS, L], F32, name="g0")
                g0_tiles[bl] = P
                # start=True clears the whole bank's has_written bits; the
                # tile spans 2 banks (2 gates each) so only g0/g2 may start
                for g in range(4):
                    nc.tensor.matmul(P[:, g], b0s_s[:, g, :], ones_s[:],
                                     start=(g % 2 == 0), stop=False,
                                     skip_group_check=True)
                    yield 1
                for k in range(KT):
                    for g in range(4):
                        nc.tensor.matmul(P[:, g], wmat[:, k, g, :], xt[:, k],
                                         start=False, stop=(k == KT - 1),
                                         skip_group_check=True)
                        yield 1

            gen = None

            def pull(n):
                nonlocal gen
                if gen is None:
                    return
                for _ in range(n):
                    if next(gen, None) is None:
                        gen = None
                        break

            def block_start(l, K):
                """State handoff into a fresh block for layer l (not block 0):
                c~ init into the scan gap slots, h0 into both ping-pong
                slot-0 columns, zero the iter-0 h-field interior."""
                nc.vector.tensor_copy(U[l][:, :, 0:1], CT[l][:, :, L:LP])
                fin = K % 2
                for p in range(2):
                    nc.vector.tensor_copy(HF[l][:, p, :, 0:1],
                                          HF[l][:, fin, :, L:LP])
                nc.vector.memset(HF[l][:, 0, :, 1:LP], 0.0)

            def iter_ops(l, P, whh_s, j, K):
                cur, nxt = j % 2, (j + 1) % 2
                rhs = HF[l][:, 0, :, 0:L] if j == 0 else DH[l][:]
                for g in range(4):
                    nc.tensor.matmul(P[:, g], whh_s[:, g, :], rhs,
                                     start=False, stop=True,
                                     skip_group_check=True)
                nc.scalar.activation(SIG[l][:, :, :, 1:LP], P[:],
                                     AF.Sigmoid)
                nc.vector.scalar_tensor_tensor(
                    U[l][:, :, 1:LP], SIG[l][:, 2, :, 1:LP], 0.5,
                    SIG[l][:, 0, :, 1:LP],
                    op0=OP.subtract, op1=OP.mult)
                nc.vector.tensor_tensor_scan(
                    CT[l][:].rearrange("p a b -> p (a b)"),
                    SIG[l][:, 1].rearrange("p a b -> p (a b)"),
                    U[l][:].rearrange("p a b -> p (a b)"),
                    0.0, op0=OP.mult, op1=OP.add)
                nc.scalar.activation(TC[l][:], CT[l][:, :, 1:LP],
                                     AF.Tanh, scale=2.0)
                nc.vector.tensor_mul(HF[l][:, nxt, :, 1:LP],
                                     SIG[l][:, 3, :, 1:LP], TC[l][:])
                if j < K - 1:
                    nc.vector.tensor_sub(DH[l][:], HF[l][:, nxt, :, 0:L],
                                         HF[l][:, cur, :, 0:L])

            def l1_xp():
                P1 = g1_pool.tile([128, 4, BS, L], F32, name="g1")
                for g in range(4):
                    nc.tensor.matmul(P1[:, g], b1s_s[:, g, :], ones_s[:],
                                     start=(g % 2 == 0), stop=False,
                                     skip_group_check=True)
                for g in range(4):
                    nc.tensor.matmul(P1[:, g], wih1t_s[:, g, :], H0FIN[:],
                                     start=False, stop=True,
                                     skip_group_check=True)
                return P1

            # ---- prologue: block-0 GEMM ----
            gen = gemm_gen(0)
            pull(KT + 2)
            _late_const_dmas()
            pull(10000)

            P1 = None
            for bl in range(NB):
                if bl > 0:
                    block_start(0, K0)
                gen = gemm_gen(bl + 1) if bl + 1 < NB else None
                for j in range(K0):
                    iter_ops(0, g0_tiles[bl], whh0t_s, j, K0)
                    if bl > 0:
                        if j == 0:
                            if bl > 1:
                                block_start(1, K1)
                            P1 = l1_xp()
                        elif j - 1 < K1:
                            iter_ops(1, P1, whh1t_s, j - 1, K1)
                    pull(18)
                pull(4)
                # final h-field of block bl -> layer-1 input
                nc.vector.tensor_copy(H0FIN[:], HF[0][:, K0 % 2, :, 1:LP])

            pull(10000)
            # layer-1 for the last block
            block_start(1, K1)
            P1 = l1_xp()
            for j in range(K1):
                iter_ops(1, P1, whh1t_s, j, K1)

            # ---- final fc ----
            fcp = g1_pool.tile([BS, 1], F32, name="fcp")
            nc.tensor.matmul(fcp[:], HF[1][:, K1 % 2, :, L:LP], wfct_s[:],
                             start=True, stop=True, skip_group_check=True)
            nc.scalar.activation(y_sb[:], fcp[:], AF.Identity,
                                 bias=bfcb_s[:])
            nc.sync.dma_start(yd[:], y_sb[:])

    nc.compile()
    return nc


_PROG = None


def _get_program():
    global _PROG
    if _PROG is None:
        _PROG = build_program()
    return _PROG


def prep_inputs(x, Wih0, Whh0, bih0, bhh0, Wih1, Whh1, bih1, bhh1, Wfc, bfc):
    """Host-side layout prep -> per-core in_maps."""
    bf = ml_dtypes.bfloat16
    x = np.asarray(x, np.float32)

    # weights: [4H, K] -> [K(part), gate, unit]; g-gate x2 (sigma(2a) trick)
    def gate_T(Wmat):
        A = np.asarray(Wmat, np.float32).reshape(4, 128, -1)  # g, j, k
        A = A.transpose(2, 0, 1).copy()                       # k, g, j
        A[:, 2, :] *= 2.0
        return np.ascontiguousarray(A)

    wih0t = gate_T(Wih0).reshape(KT, 128, 4, 128).transpose(1, 0, 2, 3)
    wih0t = np.ascontiguousarray(wih0t, np.float32)           # [128,KT,4,128]
    whh0t = gate_T(Whh0).astype(bf)                           # [128,4,128]
    whh1t = gate_T(Whh1).astype(bf)
    wih1t = gate_T(Wih1).astype(bf)

    def bias_s(ba, bb):
        b = (np.asarray(ba) + np.asarray(bb)).astype(np.float32)
        b = b.reshape(4, 128).copy()
        b[2] *= 2.0
        return b[None].astype(bf)                             # [1,4,128]

    b0s = bias_s(bih0, bhh0)
    b1s = bias_s(bih1, bhh1)
    ones = np.ones((1, NTOK), bf)
    wfct = np.asarray(Wfc, np.float32).T.astype(bf)           # [128,1]
    bfcb = np.full((BS, 1), np.asarray(bfc, np.float32)[0], np.float32)
    wih0bf = wih0t.astype(bf)

    common = dict(wih0t=wih0t, wih0bf=wih0bf,
                  whh0t=whh0t, whh1t=whh1t, wih1t=wih1t,
                  b0s=b0s, b1s=b1s, ones256=ones,
                  wfct=wfct, bfcb=bfcb)

    in_maps = []
    for c in range(NCORES):
        xc = x[c * BS:(c + 1) * BS]                           # [BS, T, D]
        xt = xc.transpose(2, 0, 1)                            # [D, BS, T]
        # [KT, 128, BS, NB, L] -> [NB, 128, KT, BS, L]
        xr = xt.reshape(KT, 128, BS, NB, L).transpose(3, 1, 0, 2, 4)
        xr = np.ascontiguousarray(xr, np.float32)
        in_maps.append({"xs": xr, "x0bf": xr[0].astype(bf), **common})
    return in_maps


def run(inputs, **kw):
    nc = _get_program()
    in_maps = prep_inputs(**inputs)
    res = run_bass_kernel_spmd(nc, in_maps, core_ids=list(range(NCORES)), **kw)
    y = np.concatenate([res.results[c]["y"] for c in range(NCORES)], axis=0)
    return y.astype(np.float32), res


def kernel(**inputs):
    y, _ = run(inputs)
    return y


if __name__ == "__main__":
    import sys
    if "--sim" in sys.argv:
        import trails.perfetto as _tp
        if not hasattr(_tp.LazyPerfetto, "add_counter"):
            def _add_counter(self, proc, track, ts_, val):
                self.update_counter(proc, track, int(ts_), float(val),
                                    unit="ns")
            _tp.LazyPerfetto.add_counter = _add_counter
        for _m in ("enable_explicit_ordering", "reserve_process_order"):
            if not hasattr(_tp.LazyPerfetto, _m):
                setattr(_tp.LazyPerfetto, _m, lambda self, *a, **k: None)
        from concourse.timeline_sim import TimelineSim
        nc = _get_program()
        ts = TimelineSim(nc, trace="--trace" in sys.argv)
        dur = ts.simulate()
        print(f"TimelineSim predicted duration: {dur:.0f} ns")
        if ts.perfetto is not None:
            ts.perfetto.save("/root/problem/timeline.pftrace")
            print("wrote /root/problem/timeline.pftrace")


# revision 3
# speedup vs baseline: 1.2594x; 1.2594x over previous
"""Trainium2 Bass kernel for a 2-layer LSTM binary classifier.

Block-parallel Picard iteration: instead of a latency-bound serial
recurrence (~1.8us/step chain), process T in blocks of L=32 steps.
Within a block, iterate K times:
  gates = xp + Whh @ h_field        (h_field = stale estimates, bf16)
  sig   = sigmoid(gates)            (one big ACT op)
  u'    = (sig_g2 - 0.5) * sig_i    (c~ = c/2 space)
  c~    = scan: c~ = sig_f * c~ + u'   (exact, tensor_tensor_scan)
  tanh_c = tanh(2*c~)               (ACT, scale=2)
  h_new = sig_o * tanh_c            (bf16)
Per iteration the gates PSUM is updated with Whh @ (h_new - h_old), so
the matmuls stay small and the PSUM accumulates the converged value.
Convergence factor ~0.1/iter (measured): K0=5 (layer0), K1=4 (layer1)
gives final rel err ~2.5e-3 (incl bf16 h-fields), vs gate 2e-2.

The per-example scan boundary is handled with a gap slot: fields have
L+1 slots per example; slot 0 carries (f=0, u=c~_init) so one
tensor_tensor_scan over the whole [128, 8*(L+1)] field resets correctly
at each example boundary and performs the cross-block state handoff.

Sharding: data-parallel over batch (64 -> 8 cores x 8 examples), all
weights replicated.  The xp GEMM (f32r; block 0 in bf16 to shorten the
DMA prologue) writes directly into each block's gates PSUM tile and is
emitted incrementally between iteration ops.  Layer 1 (lagging one
block) interleaves into layer 0's iteration slots: its K1=4 iterations
fill layer 0's K0=5 slots of the next block, so engines overlap.
"""

import numpy as np
import ml_dtypes

import concourse.bass as bass
import concourse.tile as tile
from concourse import bacc, mybir
from concourse.bass_utils import run_bass_kernel_spmd

F32 = mybir.dt.float32
F32R = mybir.dt.float32r
BF16 = mybir.dt.bfloat16
AF = mybir.ActivationFunctionType
OP = mybir.AluOpType

H = 128          # hidden
D = 2048         # input size
B = 64           # batch
T = 256          # seq len
NCORES = 8
BS = B // NCORES          # 8 examples per core
KT = D // 128             # 16 k-tiles of the input GEMM
L = 32                    # picard block length (timesteps)
NB = T // L               # 8 blocks
LP = L + 1                # field slots per example (slot 0 = gap/init)
K0 = 4                    # picard iterations, layer 0
K1 = 4                    # picard iterations, layer 1
NTOK = BS * L             # 256 tokens per block


def build_program():
    nc = bacc.Bacc("TRN2", target_bir_lowering=False, debug=False,
                   enable_asserts=False)

    # ---- DRAM I/O ----
    xsd = nc.dram_tensor("xs", [NB, 128, KT, BS, L], F32R,
                         kind="ExternalInput").ap()
    x0bfd = nc.dram_tensor("x0bf", [128, KT, BS, L], BF16,
                           kind="ExternalInput").ap()
    wih0d = nc.dram_tensor("wih0t", [128, KT, 4, 128], F32R,
                           kind="ExternalInput").ap()
    wih0bfd = nc.dram_tensor("wih0bf", [128, KT, 4, 128], BF16,
                             kind="ExternalInput").ap()
    whh0d = nc.dram_tensor("whh0t", [128, 4, 128], BF16,
                           kind="ExternalInput").ap()
    whh1d = nc.dram_tensor("whh1t", [128, 4, 128], BF16,
                           kind="ExternalInput").ap()
    wih1d = nc.dram_tensor("wih1t", [128, 4, 128], BF16,
                           kind="ExternalInput").ap()
    b0d = nc.dram_tensor("b0s", [1, 4, 128], BF16, kind="ExternalInput").ap()
    b1d = nc.dram_tensor("b1s", [1, 4, 128], BF16, kind="ExternalInput").ap()
    onesd = nc.dram_tensor("ones256", [1, NTOK], BF16,
                           kind="ExternalInput").ap()
    wfcd = nc.dram_tensor("wfct", [128, 1], BF16, kind="ExternalInput").ap()
    bfcd = nc.dram_tensor("bfcb", [BS, 1], F32, kind="ExternalInput").ap()
    yd = nc.dram_tensor("y", [BS, 1], F32, kind="ExternalOutput").ap()

    with tile.TileContext(nc) as tc, \
            tc.tile_pool(name="persist", bufs=1) as pp:
        # ---- persistent SBUF ----
        wih0t_s = pp.tile([128, KT, 4, 128], F32R, name="wih0t_s")
        wih0bf_s = pp.tile([128, KT, 4, 128], BF16, name="wih0bf_s")
        x0bf_s = pp.tile([128, KT, BS, L], BF16, name="x0bf_s")
        whh0t_s = pp.tile([128, 4, 128], BF16, name="whh0t_s")
        whh1t_s = pp.tile([128, 4, 128], BF16, name="whh1t_s")
        wih1t_s = pp.tile([128, 4, 128], BF16, name="wih1t_s")
        b0s_s = pp.tile([1, 4, 128], BF16, name="b0s_s")
        b1s_s = pp.tile([1, 4, 128], BF16, name="b1s_s")
        ones_s = pp.tile([1, NTOK], BF16, name="ones_s")
        wfct_s = pp.tile([128, 1], BF16, name="wfct_s")
        bfcb_s = pp.tile([BS, 1], F32, name="bfcb_s")
        y_sb = pp.tile([BS, 1], F32, name="y_sb")

        # picard fields: each layer has two sets (overlapping blocks):
        # layer 0 -> idx bl%2, layer 1 -> idx 2 + bl%2
        SIG = [pp.tile([128, 4, BS, LP], F32, name=f"SIG{i}") for i in range(4)]
        U = [pp.tile([128, BS, LP], F32, name=f"U{i}") for i in range(4)]
        CT = [pp.tile([128, BS, LP], F32, name=f"CT{i}") for i in range(4)]
        TC = [pp.tile([128, BS, L], F32, name=f"TC{i}") for i in range(4)]
        HF = [pp.tile([128, 2, BS, LP], BF16, name=f"HF{i}") for i in range(4)]
        DH = [pp.tile([128, BS, L], BF16, name=f"DH{i}") for i in range(4)]
        H0FIN = pp.tile([128, BS, L], BF16, name="H0FIN")

        # GEMM bias consts must precede block-0 x DMAs on the sync queue
        nc.sync.dma_start(b0s_s[:], b0d[:])
        nc.sync.dma_start(ones_s[:], onesd[:])
        for _k2 in range(0, KT, 2):
            nc.gpsimd.dma_start(wih0bf_s[:, _k2:_k2 + 2],
                                wih0bfd[:, _k2:_k2 + 2])
        for _k in range(KT):
            nc.gpsimd.dma_start(wih0t_s[:, _k], wih0d[:, _k])

        # one-time zero init: gap slots (f-gate slot0 must be exactly 0 so
        # the scan resets at example boundaries), block-0 state
        for l in range(4):
            nc.vector.memset(SIG[l][:, :, :, 0:1], 0.0)
            nc.vector.memset(U[l][:, :, 0:1], 0.0)
            nc.vector.memset(HF[l][:], 0.0)

        def _late_const_dmas():
            nc.sync.dma_start(whh0t_s[:], whh0d[:])
            nc.sync.dma_start(whh1t_s[:], whh1d[:])
            nc.sync.dma_start(wih1t_s[:], wih1d[:])
            nc.sync.dma_start(b1s_s[:], b1d[:])
            nc.sync.dma_start(wfct_s[:], wfcd[:])
            nc.sync.dma_start(bfcb_s[:], bfcd[:])

        with (
            tc.tile_pool(name="xchunk", bufs=2) as x_pool,
            tc.tile_pool(name="g0ps", bufs=2, space="PSUM") as g0_pool,
            tc.tile_pool(name="g1ps", bufs=2, space="PSUM") as g1_pool,
        ):
            g0_tiles = {}

            def gemm_gen(bl):
                """Emit block bl's xp GEMM directly into its gates PSUM."""
                if bl == 0:
                    xt, wmat = x0bf_s, wih0bf_s
                else:
                    xt = x_pool.tile([128, KT, BS, L], F32R, name="xt")
                    wmat = wih0t_s
                for k in range(KT):
                    if bl == 0:
                        nc.sync.dma_start(xt[:, k], x0bfd[:, k])
                    else:
                        nc.sync.dma_start(xt[:, k], xsd[bl, :, k])
                    yield 1
                P = g0_pool.tile([128, 4, BS, L], F32, name="g0")
                g0_tiles[bl] = P
                # start=True clears the whole bank's has_written bits; the
                # tile spans 2 banks (2 gates each) so only g0/g2 may start
                for g in range(4):
                    nc.tensor.matmul(P[:, g], b0s_s[:, g, :], ones_s[:],
                                     start=(g % 2 == 0), stop=False,
                                     skip_group_check=True)
                    yield 1
                for k in range(KT):
                    for g in range(4):
                        nc.tensor.matmul(P[:, g], wmat[:, k, g, :], xt[:, k],
                                         start=False, stop=(k == KT - 1),
                                         skip_group_check=True)
                        yield 1

            gen = None

            def pull(n):
                nonlocal gen
                if gen is None:
                    return
                for _ in range(n):
                    if next(gen, None) is None:
                        gen = None
                        break

            def block_start(idx, other, bl):
                """Speculative handoff: block bl's init state comes from the
                sibling strand's latest iterate (final state is re-copied
                during iters 1+)."""
                nc.vector.memset(HF[idx][:, 0, :, 1:LP], 0.0)
                if bl > 0:
                    nc.vector.tensor_copy(HF[idx][:, 0, :, 0:1],
                                          HF[other][:, 0, :, L:LP])
                    nc.vector.tensor_copy(U[idx][:, :, 0:1],
                                          CT[other][:, :, L:LP])

            def iter_ops(idx, other, bl, P, whh_s, j, K):
                cur, nxt = j % 2, (j + 1) % 2
                rhs = HF[idx][:, 0, :, 0:L] if j == 0 else DH[idx][:]
                for g in range(4):
                    nc.tensor.matmul(P[:, g], whh_s[:, g, :], rhs,
                                     start=False, stop=True,
                                     skip_group_check=True)
                nc.scalar.activation(SIG[idx][:, :, :, 1:LP], P[:],
                                     AF.Sigmoid)
                nc.vector.scalar_tensor_tensor(
                    U[idx][:, :, 1:LP], SIG[idx][:, 2, :, 1:LP], 0.5,
                    SIG[idx][:, 0, :, 1:LP],
                    op0=OP.subtract, op1=OP.mult)
                nc.vector.tensor_tensor_scan(
                    CT[idx][:].rearrange("p a b -> p (a b)"),
                    SIG[idx][:, 1].rearrange("p a b -> p (a b)"),
                    U[idx][:].rearrange("p a b -> p (a b)"),
                    0.0, op0=OP.mult, op1=OP.add)
                nc.scalar.activation(TC[idx][:], CT[idx][:, :, 1:LP],
                                     AF.Tanh, scale=2.0)
                nc.vector.tensor_mul(HF[idx][:, nxt, :, 1:LP],
                                     SIG[idx][:, 3, :, 1:LP], TC[idx][:])
                # refresh the speculative h0 (slot 0) from the sibling
                # strand's latest iterate, then form the delta incl the shift
                if bl > 0:
                    srcpp = 1 if j == 0 else 0
                    nc.vector.tensor_copy(HF[idx][:, nxt, :, 0:1],
                                          HF[other][:, srcpp, :, L:LP])
                    if j == 1:
                        nc.vector.tensor_copy(U[idx][:, :, 0:1],
                                              CT[other][:, :, L:LP])
                if j < K - 1:
                    nc.vector.tensor_sub(DH[idx][:], HF[idx][:, nxt, :, 0:L],
                                         HF[idx][:, cur, :, 0:L])

            def l1_xp():
                P1 = g1_pool.tile([128, 4, BS, L], F32, name="g1")
                for g in range(4):
                    nc.tensor.matmul(P1[:, g], b1s_s[:, g, :], ones_s[:],
                                     start=(g % 2 == 0), stop=False,
                                     skip_group_check=True)
                for g in range(4):
                    nc.tensor.matmul(P1[:, g], wih1t_s[:, g, :], H0FIN[:],
                                     start=False, stop=True,
                                     skip_group_check=True)
                return P1

            # ---- prologue: block-0 GEMM ----
            gen = gemm_gen(0)
            pull(KT + 2)
            _late_const_dmas()
            pull(10000)

            # ---- slot-scheduled wavefront ----
            # l0 block bl: iters j at slot 2*bl + j; blocks overlap by 2
            # iterations (a block's iters 1+ see its predecessor's final
            # state).  l1 block bl: xp at slot 2*bl+4, iters at 2*bl+5+j,
            # also overlapped by 2.
            LAGI = 2
            P1 = {}
            for s in range(LAGI * NB + 7):
                if s % LAGI == 0 and s // LAGI < NB:
                    bl = s // LAGI
                    pull(10000)          # block bl's GEMM fully emitted
                    block_start(bl % 2, 1 - bl % 2, bl)
                    gen = gemm_gen(bl + 1) if bl + 1 < NB else None
                for bl in range(max(0, (s - K0) // LAGI),
                                min(NB, s // LAGI + 1)):
                    j = s - LAGI * bl
                    if 0 <= j < K0:
                        iter_ops(bl % 2, 1 - bl % 2, bl, g0_tiles[bl],
                                 whh0t_s, j, K0)
                        if j == K0 - 1:
                            nc.vector.tensor_copy(
                                H0FIN[:], HF[bl % 2][:, K0 % 2, :, 1:LP])
                for bl in range(NB):
                    i1, o1 = 2 + bl % 2, 2 + (1 - bl % 2)
                    if s == LAGI * bl + 4:
                        block_start(i1, o1, bl)
                        P1[bl] = l1_xp()
                    j1 = s - (LAGI * bl + 5)
                    if 0 <= j1 < K1:
                        iter_ops(i1, o1, bl, P1[bl], whh1t_s, j1, K1)
                pull(45)

            # ---- final fc (reuses a corner of the last gates PSUM tile) ----
            fcp = g0_tiles[NB - 1][0:BS, 0, 0, 0:1]
            nc.tensor.matmul(fcp, HF[2 + (NB - 1) % 2][:, K1 % 2, :, L:LP],
                             wfct_s[:],
                             start=True, stop=True, skip_group_check=True)
            nc.scalar.activation(y_sb[:], fcp, AF.Identity,
                                 bias=bfcb_s[:])
            nc.sync.dma_start(yd[:], y_sb[:])

    nc.compile()
    return nc


_PROG = None


def _get_program():
    global _PROG
    if _PROG is None:
        _PROG = build_program()
    return _PROG


def prep_inputs(x, Wih0, Whh0, bih0, bhh0, Wih1, Whh1, bih1, bhh1, Wfc, bfc):
    """Host-side layout prep -> per-core in_maps."""
    bf = ml_dtypes.bfloat16
    x = np.asarray(x, np.float32)

    # weights: [4H, K] -> [K(part), gate, unit]; g-gate x2 (sigma(2a) trick)
    def gate_T(Wmat):
        A = np.asarray(Wmat, np.float32).reshape(4, 128, -1)  # g, j, k
        A = A.transpose(2, 0, 1).copy()                       # k, g, j
        A[:, 2, :] *= 2.0
        return np.ascontiguousarray(A)

    wih0t = gate_T(Wih0).reshape(KT, 128, 4, 128).transpose(1, 0, 2, 3)
    wih0t = np.ascontiguousarray(wih0t, np.float32)           # [128,KT,4,128]
    whh0t = gate_T(Whh0).astype(bf)                           # [128,4,128]
    whh1t = gate_T(Whh1).astype(bf)
    wih1t = gate_T(Wih1).astype(bf)

    def bias_s(ba, bb):
        b = (np.asarray(ba) + np.asarray(bb)).astype(np.float32)
        b = b.reshape(4, 128).copy()
        b[2] *= 2.0
        return b[None].astype(bf)                             # [1,4,128]

    b0s = bias_s(bih0, bhh0)
    b1s = bias_s(bih1, bhh1)
    ones = np.ones((1, NTOK), bf)
    wfct = np.asarray(Wfc, np.float32).T.astype(bf)           # [128,1]
    bfcb = np.full((BS, 1), np.asarray(bfc, np.float32)[0], np.float32)
    wih0bf = wih0t.astype(bf)

    common = dict(wih0t=wih0t, wih0bf=wih0bf,
                  whh0t=whh0t, whh1t=whh1t, wih1t=wih1t,
                  b0s=b0s, b1s=b1s, ones256=ones,
                  wfct=wfct, bfcb=bfcb)

    in_maps = []
    for c in range(NCORES):
        xc = x[c * BS:(c + 1) * BS]                           # [BS, T, D]
        xt = xc.transpose(2, 0, 1)                            # [D, BS, T]
        # [KT, 128, BS, NB, L] -> [NB, 128, KT, BS, L]
        xr = xt.reshape(KT, 128, BS, NB, L).transpose(3, 1, 0, 2, 4)
        xr = np.ascontiguousarray(xr, np.float32)
        in_maps.append({"xs": xr, "x0bf": xr[0].astype(bf), **common})
    return in_maps


def run(inputs, **kw):
    nc = _get_program()
    in_maps = prep_inputs(**inputs)
    res = run_bass_kernel_spmd(nc, in_maps, core_ids=list(range(NCORES)), **kw)
    y = np.concatenate([res.results[c]["y"] for c in range(NCORES)], axis=0)
    return y.astype(np.float32), res


def kernel(**inputs):
    y, _ = run(inputs)
    return y


if __name__ == "__main__":
    import sys
    if "--sim" in sys.argv:
        import trails.perfetto as _tp
        if not hasattr(_tp.LazyPerfetto, "add_counter"):
            def _add_counter(self, proc, track, ts_, val):
                self.update_counter(proc, track, int(ts_), float(val),
                                    unit="ns")
            _tp.LazyPerfetto.add_counter = _add_counter
        for _m in ("enable_explicit_ordering", "reserve_process_order"):
            if not hasattr(_tp.LazyPerfetto, _m):
                setattr(_tp.LazyPerfetto, _m, lambda self, *a, **k: None)
        from concourse.timeline_sim import TimelineSim
        nc = _get_program()
        ts = TimelineSim(nc, trace="--trace" in sys.argv)
        dur = ts.simulate()
        print(f"TimelineSim predicted duration: {dur:.0f} ns")
        if ts.perfetto is not None:
            ts.perfetto.save("/root/problem/timeline.pftrace")
            print("wrote /root/problem/timeline.pftrace")


# revision 7
# speedup vs baseline: 1.5950x; 1.2665x over previous
"""Trainium2 Bass kernel for a 2-layer LSTM binary classifier.

Block-parallel Picard iteration: instead of a latency-bound serial
recurrence (~1.8us/step chain), process T in blocks of L=32 steps.
Within a block, iterate K times:
  gates = xp + Whh @ h_field        (h_field = stale estimates, bf16)
  sig   = sigmoid(gates)            (one big ACT op)
  u'    = (sig_g2 - 0.5) * sig_i    (c~ = c/2 space)
  c~    = scan: c~ = sig_f * c~ + u'   (exact, tensor_tensor_scan)
  tanh_c = tanh(2*c~)               (ACT, scale=2)
  h_new = sig_o * tanh_c            (bf16)
Per iteration the gates PSUM is updated with Whh @ (h_new - h_old), so
the matmuls stay small and the PSUM accumulates the converged value.
Convergence factor ~0.1/iter (measured): K0=K1=4 iterations give
final rel err ~2.2e-3 (incl bf16 h-fields), vs the 2e-2 gate.

The per-example scan boundary is handled with a gap slot: fields have
L+1 slots per example; slot 0 carries (f=0, u=c~_init) so one
tensor_tensor_scan over the whole [128, 8*(L+1)] field resets correctly
at each example boundary and performs the cross-block state handoff.

Sharding: data-parallel over batch (64 -> 8 cores x 8 examples), all
weights replicated.  The xp GEMM (all-bf16; one batched HWDGE DMA per
block) writes directly into each block's gates PSUM tile and is
emitted incrementally between iteration ops.  Both layers' blocks are
slot-scheduled as overlapping speculative strands (a block starts two
iteration-slots before its predecessor finishes, re-copying the
predecessor's latest h0/c~ handoff each iteration; iters 1+ see the
converged state), so the dependency wall shrinks toward engine
capacity.
"""

import numpy as np
import ml_dtypes

import concourse.bass as bass
import concourse.tile as tile
from concourse import bacc, mybir
from concourse.bass_utils import run_bass_kernel_spmd

F32 = mybir.dt.float32
F32R = mybir.dt.float32r
BF16 = mybir.dt.bfloat16
AF = mybir.ActivationFunctionType
OP = mybir.AluOpType

H = 128          # hidden
D = 2048         # input size
B = 64           # batch
T = 256          # seq len
NCORES = 8
BS = B // NCORES          # 8 examples per core
KT = D // 128             # 16 k-tiles of the input GEMM
L = 32                    # picard block length (timesteps)
NB = T // L               # 8 blocks
LP = L + 1                # field slots per example (slot 0 = gap/init)
K0 = 3                    # picard iterations, layer 0
K1 = 4                    # picard iterations, layer 1
NTOK = BS * L             # 256 tokens per block


def build_program():
    nc = bacc.Bacc("TRN2", target_bir_lowering=False, debug=False,
                   enable_asserts=False)

    # ---- DRAM I/O ----
    xsbd = nc.dram_tensor("xsb", [NB, 128, KT, BS, L], BF16,
                          kind="ExternalInput").ap()
    wih0bfd = nc.dram_tensor("wih0bf", [128, KT, 4, 128], BF16,
                             kind="ExternalInput").ap()
    whh0d = nc.dram_tensor("whh0t", [128, 4, 128], BF16,
                           kind="ExternalInput").ap()
    whh1d = nc.dram_tensor("whh1t", [128, 4, 128], BF16,
                           kind="ExternalInput").ap()
    wih1d = nc.dram_tensor("wih1t", [128, 4, 128], BF16,
                           kind="ExternalInput").ap()
    b0d = nc.dram_tensor("b0s", [1, 4, 128], BF16, kind="ExternalInput").ap()
    b1d = nc.dram_tensor("b1s", [1, 4, 128], BF16, kind="ExternalInput").ap()
    onesd = nc.dram_tensor("ones256", [1, NTOK], BF16,
                           kind="ExternalInput").ap()
    wfcd = nc.dram_tensor("wfct", [128, 1], BF16, kind="ExternalInput").ap()
    bfcd = nc.dram_tensor("bfcb", [BS, 1], F32, kind="ExternalInput").ap()
    yd = nc.dram_tensor("y", [BS, 1], F32, kind="ExternalOutput").ap()

    with tile.TileContext(nc) as tc, \
            tc.tile_pool(name="persist", bufs=1) as pp:
        # ---- persistent SBUF ----
        wih0bf_s = pp.tile([128, KT, 4, 128], BF16, name="wih0bf_s")
        whh0t_s = pp.tile([128, 4, 128], BF16, name="whh0t_s")
        whh1t_s = pp.tile([128, 4, 128], BF16, name="whh1t_s")
        wih1t_s = pp.tile([128, 4, 128], BF16, name="wih1t_s")
        b0s_s = pp.tile([1, 4, 128], BF16, name="b0s_s")
        b1s_s = pp.tile([1, 4, 128], BF16, name="b1s_s")
        ones_s = pp.tile([1, NTOK], BF16, name="ones_s")
        wfct_s = pp.tile([128, 1], BF16, name="wfct_s")
        bfcb_s = pp.tile([BS, 1], F32, name="bfcb_s")
        y_sb = pp.tile([BS, 1], F32, name="y_sb")

        # picard fields: each layer has two sets (overlapping blocks):
        # layer 0 -> idx bl%2, layer 1 -> idx 2 + bl%2
        SIG = [pp.tile([128, 4, BS, LP], F32, name=f"SIG{i}") for i in range(4)]
        U = [pp.tile([128, BS, LP], F32, name=f"U{i}") for i in range(4)]
        CT = [pp.tile([128, BS, LP], F32, name=f"CT{i}") for i in range(4)]
        TC = [pp.tile([128, BS, L], F32, name=f"TC{i}") for i in range(4)]
        HF = [pp.tile([128, 2, BS, LP], BF16, name=f"HF{i}") for i in range(4)]
        DH = [pp.tile([128, BS, L], BF16, name=f"DH{i}") for i in range(4)]
        H0FIN = pp.tile([128, BS, L], BF16, name="H0FIN")

        # GEMM bias consts must precede block-0 x DMAs on the sync queue
        nc.sync.dma_start(b0s_s[:], b0d[:])
        nc.sync.dma_start(ones_s[:], onesd[:])

        # one-time zero init: gap slots (f-gate slot0 must be exactly 0 so
        # the scan resets at example boundaries), block-0 state
        for l in range(4):
            nc.vector.memset(SIG[l][:, :, :, 0:1], 0.0)
            nc.vector.memset(U[l][:, :, 0:1], 0.0)
            nc.vector.memset(HF[l][:], 0.0)

        def _late_const_dmas():
            # on the gpsimd/SWDGE queue: Pool is otherwise idle, keeping
            # these small consts off the SP/ACT queues that carry the
            # prologue x/weight slabs (each dma_start costs ~0.6us of its
            # queue's sequencer + HWDGE slot)
            nc.gpsimd.dma_start(whh0t_s[:], whh0d[:])
            nc.gpsimd.dma_start(whh1t_s[:], whh1d[:])
            nc.gpsimd.dma_start(wih1t_s[:], wih1d[:])
            nc.gpsimd.dma_start(b1s_s[:], b1d[:])
            nc.gpsimd.dma_start(wfct_s[:], wfcd[:])
            nc.gpsimd.dma_start(bfcb_s[:], bfcd[:])

        with (
            tc.tile_pool(name="xchunk", bufs=2) as x_pool,
            tc.tile_pool(name="g0ps", bufs=2, space="PSUM") as g0_pool,
            tc.tile_pool(name="g1ps", bufs=2, space="PSUM") as g1_pool,
        ):
            g0_tiles = {}

            def gemm_gen(bl):
                """Emit block bl's xp GEMM directly into its gates PSUM."""
                xt = x_pool.tile([128, KT, BS, L], BF16, name="xt")
                if bl == 0:
                    # interleave x / weight quarter-slabs (x on the SP queue,
                    # weights on the ACT queue) so block-0 k-matmuls start as
                    # soon as their k-range has landed
                    for q in range(0, KT, 4):
                        nc.sync.dma_start(xt[:, q:q + 4], xsbd[0, :, q:q + 4])
                        nc.scalar.dma_start(wih0bf_s[:, q:q + 4],
                                            wih0bfd[:, q:q + 4])
                        if q == 0:
                            _late_const_dmas()
                        yield 1
                else:
                    nc.sync.dma_start(xt[:], xsbd[bl])
                    yield 1
                P = g0_pool.tile([128, 4, BS, L], F32, name="g0")
                g0_tiles[bl] = P
                # start=True clears the whole bank's has_written bits; the
                # tile spans 2 banks (2 gates each) so only g0/g2 may start
                for g in range(4):
                    nc.tensor.matmul(P[:, g], b0s_s[:, g, :], ones_s[:],
                                     start=(g % 2 == 0), stop=False,
                                     skip_group_check=True)
                    yield 1
                for k in range(KT):
                    for g in range(4):
                        nc.tensor.matmul(P[:, g], wih0bf_s[:, k, g, :],
                                         xt[:, k],
                                         start=False, stop=(k == KT - 1),
                                         skip_group_check=True)
                        yield 1

            gen = None

            def pull(n):
                nonlocal gen
                if gen is None:
                    return
                for _ in range(n):
                    if next(gen, None) is None:
                        gen = None
                        break

            def block_start(idx, other, bl):
                """Speculative handoff: block bl's init state comes from the
                sibling strand's latest iterate (final state is re-copied
                during iters 1+)."""
                nc.vector.memset(HF[idx][:, 0, :, 1:LP], 0.0)
                if bl > 0:
                    nc.vector.tensor_copy(HF[idx][:, 0, :, 0:1],
                                          HF[other][:, 0, :, L:LP])
                    nc.vector.tensor_copy(U[idx][:, :, 0:1],
                                          CT[other][:, :, L:LP])

            def iter_ops(idx, other, bl, P, whh_s, j, K):
                cur, nxt = j % 2, (j + 1) % 2
                # refresh the speculative h0 (slot 0) / c~ init from the
                # sibling strand's latest iterate; emitted ahead of the
                # chain (only WAR-ordered against iter j-1) so it runs
                # during the matmul/sigmoid wait
                if bl > 0:
                    srcpp = (min(j + 2, K - 1) + 1) % 2
                    nc.vector.tensor_copy(HF[idx][:, nxt, :, 0:1],
                                          HF[other][:, srcpp, :, L:LP])
                    if j == 1:
                        nc.vector.tensor_copy(U[idx][:, :, 0:1],
                                              CT[other][:, :, L:LP])
                rhs = HF[idx][:, 0, :, 0:L] if j == 0 else DH[idx][:]
                for g in range(4):
                    nc.tensor.matmul(P[:, g], whh_s[:, g, :], rhs,
                                     start=False, stop=True,
                                     skip_group_check=True)
                nc.scalar.activation(SIG[idx][:, 0:3, :, 1:LP], P[:, 0:3],
                                     AF.Sigmoid)
                nc.scalar.activation(SIG[idx][:, 3, :, 1:LP], P[:, 3],
                                     AF.Sigmoid)
                nc.vector.scalar_tensor_tensor(
                    U[idx][:, :, 1:LP], SIG[idx][:, 2, :, 1:LP], 0.5,
                    SIG[idx][:, 0, :, 1:LP],
                    op0=OP.subtract, op1=OP.mult)
                nc.vector.tensor_tensor_scan(
                    CT[idx][:].rearrange("p a b -> p (a b)"),
                    SIG[idx][:, 1].rearrange("p a b -> p (a b)"),
                    U[idx][:].rearrange("p a b -> p (a b)"),
                    0.0, op0=OP.mult, op1=OP.add)
                nc.scalar.activation(TC[idx][:], CT[idx][:, :, 1:LP],
                                     AF.Tanh, scale=2.0)
                nc.vector.tensor_mul(HF[idx][:, nxt, :, 1:LP],
                                     SIG[idx][:, 3, :, 1:LP], TC[idx][:])
                if j < K - 1:
                    nc.vector.tensor_sub(DH[idx][:], HF[idx][:, nxt, :, 0:L],
                                         HF[idx][:, cur, :, 0:L])

            def l1_xp():
                P1 = g1_pool.tile([128, 4, BS, L], F32, name="g1")
                for g in range(4):
                    nc.tensor.matmul(P1[:, g], b1s_s[:, g, :], ones_s[:],
                                     start=(g % 2 == 0), stop=False,
                                     skip_group_check=True)
                for g in range(4):
                    nc.tensor.matmul(P1[:, g], wih1t_s[:, g, :], H0FIN[:],
                                     start=False, stop=True,
                                     skip_group_check=True)
                return P1

            # ---- prologue: block-0 GEMM ----
            gen = gemm_gen(0)
            pull(10000)

            # ---- slot-scheduled wavefront ----
            # l0 block bl: iters j at slot 2*bl + j; blocks overlap by 2
            # iterations (a block's iters 1+ see its predecessor's final
            # state).  l1 block bl: xp at slot 2*bl+4, iters at 2*bl+5+j,
            # also overlapped by 2.
            LAGI = 2
            P1 = {}
            for s in range(LAGI * NB + 7):
                if s % LAGI == 0 and s // LAGI < NB:
                    bl = s // LAGI
                    pull(10000)          # block bl's GEMM fully emitted
                    block_start(bl % 2, 1 - bl % 2, bl)
                    gen = gemm_gen(bl + 1) if bl + 1 < NB else None
                for bl in range(max(0, (s - K0) // LAGI),
                                min(NB, s // LAGI + 1)):
                    j = s - LAGI * bl
                    if 0 <= j < K0:
                        iter_ops(bl % 2, 1 - bl % 2, bl, g0_tiles[bl],
                                 whh0t_s, j, K0)
                        if j == K0 - 1:
                            nc.vector.tensor_copy(
                                H0FIN[:], HF[bl % 2][:, K0 % 2, :, 1:LP])
                for bl in range(NB):
                    i1, o1 = 2 + bl % 2, 2 + (1 - bl % 2)
                    if s == LAGI * bl + 3:
                        block_start(i1, o1, bl)
                        P1[bl] = l1_xp()
                    j1 = s - (LAGI * bl + 4)
                    if 0 <= j1 < K1:
                        iter_ops(i1, o1, bl, P1[bl], whh1t_s, j1, K1)
                pull(40)

            # ---- final fc (reuses a corner of the last gates PSUM tile) ----
            fcp = g0_tiles[NB - 1][0:BS, 0, 0, 0:1]
            nc.tensor.matmul(fcp, HF[2 + (NB - 1) % 2][:, K1 % 2, :, L:LP],
                             wfct_s[:],
                             start=True, stop=True, skip_group_check=True)
            nc.scalar.activation(y_sb[:], fcp, AF.Identity,
                                 bias=bfcb_s[:])
            nc.sync.dma_start(yd[:], y_sb[:])

    nc.compile()
    return nc


_PROG = None


def _get_program():
    global _PROG
    if _PROG is None:
        _PROG = build_program()
    return _PROG


def prep_inputs(x, Wih0, Whh0, bih0, bhh0, Wih1, Whh1, bih1, bhh1, Wfc, bfc):
    """Host-side layout prep -> per-core in_maps."""
    bf = ml_dtypes.bfloat16
    x = np.asarray(x, np.float32)

    # weights: [4H, K] -> [K(part), gate, unit]; g-gate x2 (sigma(2a) trick)
    def gate_T(Wmat):
        A = np.asarray(Wmat, np.float32).reshape(4, 128, -1)  # g, j, k
        A = A.transpose(2, 0, 1).copy()                       # k, g, j
        A[:, 2, :] *= 2.0
        return np.ascontiguousarray(A)

    wih0t = gate_T(Wih0).reshape(KT, 128, 4, 128).transpose(1, 0, 2, 3)
    wih0bf = np.ascontiguousarray(wih0t).astype(bf)           # [128,KT,4,128]
    whh0t = gate_T(Whh0).astype(bf)                           # [128,4,128]
    whh1t = gate_T(Whh1).astype(bf)
    wih1t = gate_T(Wih1).astype(bf)

    def bias_s(ba, bb):
        b = (np.asarray(ba) + np.asarray(bb)).astype(np.float32)
        b = b.reshape(4, 128).copy()
        b[2] *= 2.0
        return b[None].astype(bf)                             # [1,4,128]

    b0s = bias_s(bih0, bhh0)
    b1s = bias_s(bih1, bhh1)
    ones = np.ones((1, NTOK), bf)
    wfct = np.asarray(Wfc, np.float32).T.astype(bf)           # [128,1]
    bfcb = np.full((BS, 1), np.asarray(bfc, np.float32)[0], np.float32)
    common = dict(wih0bf=wih0bf,
                  whh0t=whh0t, whh1t=whh1t, wih1t=wih1t,
                  b0s=b0s, b1s=b1s, ones256=ones,
                  wfct=wfct, bfcb=bfcb)

    in_maps = []
    for c in range(NCORES):
        xc = x[c * BS:(c + 1) * BS]                           # [BS, T, D]
        xt = xc.transpose(2, 0, 1)                            # [D, BS, T]
        # [KT, 128, BS, NB, L] -> [NB, 128, KT, BS, L]
        xr = xt.reshape(KT, 128, BS, NB, L).transpose(3, 1, 0, 2, 4)
        in_maps.append({"xsb": np.ascontiguousarray(xr).astype(bf), **common})
    return in_maps


def run(inputs, **kw):
    nc = _get_program()
    in_maps = prep_inputs(**inputs)
    res = run_bass_kernel_spmd(nc, in_maps, core_ids=list(range(NCORES)), **kw)
    y = np.concatenate([res.results[c]["y"] for c in range(NCORES)], axis=0)
    return y.astype(np.float32), res


def kernel(**inputs):
    y, _ = run(inputs)
    return y


if __name__ == "__main__":
    import sys
    if "--sim" in sys.argv:
        import trails.perfetto as _tp
        if not hasattr(_tp.LazyPerfetto, "add_counter"):
            def _add_counter(self, proc, track, ts_, val):
                self.update_counter(proc, track, int(ts_), float(val),
                                    unit="ns")
            _tp.LazyPerfetto.add_counter = _add_counter
        for _m in ("enable_explicit_ordering", "reserve_process_order"):
            if not hasattr(_tp.LazyPerfetto, _m):
                setattr(_tp.LazyPerfetto, _m, lambda self, *a, **k: None)
        from concourse.timeline_sim import TimelineSim
        nc = _get_program()
        ts = TimelineSim(nc, trace="--trace" in sys.argv)
        dur = ts.simulate()
        print(f"TimelineSim predicted duration: {dur:.0f} ns")
        if ts.perfetto is not None:
            ts.perfetto.save("/root/problem/timeline.pftrace")
            print("wrote /root/problem/timeline.pftrace")


# revision 8
# speedup vs baseline: 1.6713x; 1.0478x over previous
"""Trainium2 Bass kernel for a 2-layer LSTM binary classifier.

Block-parallel Picard iteration: instead of a latency-bound serial
recurrence (~1.8us/step chain), process T in blocks of L=32 steps.
Within a block, iterate K times:
  gates = xp + Whh @ h_field        (h_field = stale estimates, bf16)
  sig   = sigmoid(gates)            (one big ACT op)
  u'    = (sig_g2 - 0.5) * sig_i    (c~ = c/2 space)
  c~    = scan: c~ = sig_f * c~ + u'   (exact, tensor_tensor_scan)
  tanh_c = tanh(2*c~)               (ACT, scale=2)
  h_new = sig_o * tanh_c            (bf16)
Per iteration the gates PSUM is updated with Whh @ (h_new - h_old), so
the matmuls stay small and the PSUM accumulates the converged value.
Convergence factor ~0.1/iter (measured): K0=K1=4 iterations give
final rel err ~2.2e-3 (incl bf16 h-fields), vs the 2e-2 gate.

The per-example scan boundary is handled with a gap slot: fields have
L+1 slots per example; slot 0 carries (f=0, u=c~_init) so one
tensor_tensor_scan over the whole [128, 8*(L+1)] field resets correctly
at each example boundary and performs the cross-block state handoff.

Sharding: data-parallel over batch (64 -> 8 cores x 8 examples), all
weights replicated.  The xp GEMM (all-bf16; one batched HWDGE DMA per
block) writes directly into each block's gates PSUM tile and is
emitted incrementally between iteration ops.  Both layers' blocks are
slot-scheduled as overlapping speculative strands (a block starts two
iteration-slots before its predecessor finishes, re-copying the
predecessor's latest h0/c~ handoff each iteration; iters 1+ see the
converged state), so the dependency wall shrinks toward engine
capacity.
"""

import numpy as np
import ml_dtypes

import concourse.bass as bass
import concourse.tile as tile
from concourse import bacc, mybir
from concourse.bass_utils import run_bass_kernel_spmd

F32 = mybir.dt.float32
F32R = mybir.dt.float32r
BF16 = mybir.dt.bfloat16
AF = mybir.ActivationFunctionType
OP = mybir.AluOpType

H = 128          # hidden
D = 2048         # input size
B = 64           # batch
T = 256          # seq len
NCORES = 8
BS = B // NCORES          # 8 examples per core
KT = D // 128             # 16 k-tiles of the input GEMM
L = 32                    # picard block length (timesteps)
NB = T // L               # 8 blocks
LP = L + 1                # field slots per example (slot 0 = gap/init)
K0 = 3                    # picard iterations, layer 0
K1 = 4                    # picard iterations, layer 1
NTOK = BS * L             # 256 tokens per block


def build_program():
    nc = bacc.Bacc("TRN2", target_bir_lowering=False, debug=False,
                   enable_asserts=False)

    # ---- DRAM I/O ----
    xsbd = nc.dram_tensor("xsb", [NB, 128, KT, BS, L], BF16,
                          kind="ExternalInput").ap()
    wih0bfd = nc.dram_tensor("wih0bf", [128, KT, 4, 128], BF16,
                             kind="ExternalInput").ap()
    whh0d = nc.dram_tensor("whh0t", [128, 4, 128], BF16,
                           kind="ExternalInput").ap()
    whh1d = nc.dram_tensor("whh1t", [128, 4, 128], BF16,
                           kind="ExternalInput").ap()
    wih1d = nc.dram_tensor("wih1t", [128, 4, 128], BF16,
                           kind="ExternalInput").ap()
    b0d = nc.dram_tensor("b0s", [1, 4, 128], BF16, kind="ExternalInput").ap()
    b1d = nc.dram_tensor("b1s", [1, 4, 128], BF16, kind="ExternalInput").ap()
    onesd = nc.dram_tensor("ones256", [1, NTOK], BF16,
                           kind="ExternalInput").ap()
    wfcd = nc.dram_tensor("wfct", [128, 1], BF16, kind="ExternalInput").ap()
    bfcd = nc.dram_tensor("bfcb", [BS, 1], F32, kind="ExternalInput").ap()
    yd = nc.dram_tensor("y", [BS, 1], F32, kind="ExternalOutput").ap()

    with tile.TileContext(nc) as tc, \
            tc.tile_pool(name="persist", bufs=1) as pp:
        # ---- persistent SBUF ----
        wih0bf_s = pp.tile([128, KT, 4, 128], BF16, name="wih0bf_s")
        whh0t_s = pp.tile([128, 4, 128], BF16, name="whh0t_s")
        whh1t_s = pp.tile([128, 4, 128], BF16, name="whh1t_s")
        wih1t_s = pp.tile([128, 4, 128], BF16, name="wih1t_s")
        b0s_s = pp.tile([1, 4, 128], BF16, name="b0s_s")
        b1s_s = pp.tile([1, 4, 128], BF16, name="b1s_s")
        ones_s = pp.tile([1, NTOK], BF16, name="ones_s")
        wfct_s = pp.tile([128, 1], BF16, name="wfct_s")
        bfcb_s = pp.tile([BS, 1], F32, name="bfcb_s")
        y_sb = pp.tile([BS, 1], F32, name="y_sb")

        # picard fields: each layer has two sets (overlapping blocks):
        # layer 0 -> idx bl%2, layer 1 -> idx 2 + bl%2
        SIG = [pp.tile([128, 4, BS, LP], F32, name=f"SIG{i}") for i in range(5)]
        U = [pp.tile([128, BS, LP], F32, name=f"U{i}") for i in range(5)]
        CT = [pp.tile([128, BS, LP], F32, name=f"CT{i}") for i in range(5)]
        TC = [pp.tile([128, BS, L], F32, name=f"TC{i}") for i in range(5)]
        HF = [pp.tile([128, 2, BS, LP], BF16, name=f"HF{i}") for i in range(5)]
        DH = [pp.tile([128, BS, L], BF16, name=f"DH{i}") for i in range(5)]
        H0FIN = pp.tile([128, BS, L], BF16, name="H0FIN")
        H0STALE = pp.tile([128, BS, L], BF16, name="H0STALE")
        DHX = pp.tile([128, BS, L], BF16, name="DHX")

        # GEMM bias consts must precede block-0 x DMAs on the sync queue
        nc.sync.dma_start(b0s_s[:], b0d[:])
        nc.sync.dma_start(ones_s[:], onesd[:])

        # one-time zero init: gap slots (f-gate slot0 must be exactly 0 so
        # the scan resets at example boundaries), block-0 state
        for l in range(5):
            nc.vector.memset(SIG[l][:, :, :, 0:1], 0.0)
            nc.vector.memset(U[l][:, :, 0:1], 0.0)
            nc.vector.memset(HF[l][:], 0.0)

        def _late_const_dmas():
            # on the gpsimd/SWDGE queue: Pool is otherwise idle, keeping
            # these small consts off the SP/ACT queues that carry the
            # prologue x/weight slabs (each dma_start costs ~0.6us of its
            # queue's sequencer + HWDGE slot)
            nc.gpsimd.dma_start(whh0t_s[:], whh0d[:])
            nc.gpsimd.dma_start(whh1t_s[:], whh1d[:])
            nc.gpsimd.dma_start(wih1t_s[:], wih1d[:])
            nc.gpsimd.dma_start(b1s_s[:], b1d[:])
            nc.gpsimd.dma_start(wfct_s[:], wfcd[:])
            nc.gpsimd.dma_start(bfcb_s[:], bfcd[:])

        with (
            tc.tile_pool(name="xchunk", bufs=2) as x_pool,
            tc.tile_pool(name="g0ps", bufs=2, space="PSUM") as g0_pool,
            tc.tile_pool(name="g1ps", bufs=2, space="PSUM") as g1_pool,
        ):
            g0_tiles = {}

            def gemm_gen(bl):
                """Emit block bl's xp GEMM directly into its gates PSUM."""
                xt = x_pool.tile([128, KT, BS, L], BF16, name="xt")
                if bl == 0:
                    # interleave x / weight quarter-slabs (x on the SP queue,
                    # weights on the ACT queue) so block-0 k-matmuls start as
                    # soon as their k-range has landed
                    for q in range(0, KT, 4):
                        nc.sync.dma_start(xt[:, q:q + 4], xsbd[0, :, q:q + 4])
                        nc.scalar.dma_start(wih0bf_s[:, q:q + 4],
                                            wih0bfd[:, q:q + 4])
                        if q == 0:
                            _late_const_dmas()
                        yield 1
                else:
                    nc.sync.dma_start(xt[:], xsbd[bl])
                    yield 1
                P = g0_pool.tile([128, 4, BS, L], F32, name="g0")
                g0_tiles[bl] = P
                # start=True clears the whole bank's has_written bits; the
                # tile spans 2 banks (2 gates each) so only g0/g2 may start
                for g in range(4):
                    nc.tensor.matmul(P[:, g], b0s_s[:, g, :], ones_s[:],
                                     start=(g % 2 == 0), stop=False,
                                     skip_group_check=True)
                    yield 1
                for k in range(KT):
                    for g in range(4):
                        nc.tensor.matmul(P[:, g], wih0bf_s[:, k, g, :],
                                         xt[:, k],
                                         start=False, stop=(k == KT - 1),
                                         skip_group_check=True)
                        yield 1

            gen = None

            def pull(n):
                nonlocal gen
                if gen is None:
                    return
                for _ in range(n):
                    if next(gen, None) is None:
                        gen = None
                        break

            def block_start(idx, other, bl):
                """Speculative handoff: block bl's init state comes from the
                sibling strand's latest iterate (final state is re-copied
                during iters 1+)."""
                nc.vector.memset(HF[idx][:, 0, :, 1:LP], 0.0)
                if bl > 0:
                    nc.vector.tensor_copy(HF[idx][:, 0, :, 0:1],
                                          HF[other][:, 0, :, L:LP])
                    nc.vector.tensor_copy(U[idx][:, :, 0:1],
                                          CT[other][:, :, L:LP])

            def iter_ops(idx, other, bl, P, whh_s, j, K):
                cur, nxt = j % 2, (j + 1) % 2
                # refresh the speculative h0 (slot 0) / c~ init from the
                # sibling strand's latest iterate; emitted ahead of the
                # chain (only WAR-ordered against iter j-1) so it runs
                # during the matmul/sigmoid wait
                if bl > 0:
                    srcpp = (min(j + 2, K - 1) + 1) % 2
                    nc.vector.tensor_copy(HF[idx][:, nxt, :, 0:1],
                                          HF[other][:, srcpp, :, L:LP])
                    if j >= 1:
                        nc.vector.tensor_copy(U[idx][:, :, 0:1],
                                              CT[other][:, :, L:LP])
                rhs = HF[idx][:, 0, :, 0:L] if j == 0 else DH[idx][:]
                for g in range(4):
                    nc.tensor.matmul(P[:, g], whh_s[:, g, :], rhs,
                                     start=False, stop=True,
                                     skip_group_check=True)
                nc.scalar.activation(SIG[idx][:, 0:3, :, 1:LP], P[:, 0:3],
                                     AF.Sigmoid)
                nc.scalar.activation(SIG[idx][:, 3, :, 1:LP], P[:, 3],
                                     AF.Sigmoid)
                nc.vector.scalar_tensor_tensor(
                    U[idx][:, :, 1:LP], SIG[idx][:, 2, :, 1:LP], 0.5,
                    SIG[idx][:, 0, :, 1:LP],
                    op0=OP.subtract, op1=OP.mult)
                nc.vector.tensor_tensor_scan(
                    CT[idx][:].rearrange("p a b -> p (a b)"),
                    SIG[idx][:, 1].rearrange("p a b -> p (a b)"),
                    U[idx][:].rearrange("p a b -> p (a b)"),
                    0.0, op0=OP.mult, op1=OP.add)
                nc.scalar.activation(TC[idx][:], CT[idx][:, :, 1:LP],
                                     AF.Tanh, scale=2.0)
                nc.vector.tensor_mul(HF[idx][:, nxt, :, 1:LP],
                                     SIG[idx][:, 3, :, 1:LP], TC[idx][:])
                if j < K - 1:
                    nc.vector.tensor_sub(DH[idx][:], HF[idx][:, nxt, :, 0:L],
                                         HF[idx][:, cur, :, 0:L])

            def l1_xp(hsrc, from_g0=False):
                pool = g0_pool if from_g0 else g1_pool
                P1 = pool.tile([128, 4, BS, L], F32,
                               name="g0" if from_g0 else "g1")
                for g in range(4):
                    nc.tensor.matmul(P1[:, g], b1s_s[:, g, :], ones_s[:],
                                     start=(g % 2 == 0), stop=False,
                                     skip_group_check=True)
                for g in range(4):
                    nc.tensor.matmul(P1[:, g], wih1t_s[:, g, :], hsrc[:],
                                     start=False, stop=True,
                                     skip_group_check=True)
                return P1

            # ---- prologue: block-0 GEMM ----
            gen = gemm_gen(0)
            pull(10000)

            # ---- slot-scheduled wavefront ----
            # l0 block bl: iters j at slot 2*bl + j; blocks overlap by 2
            # iterations (a block's iters 1+ see its predecessor's final
            # state).  l1 block bl: xp at slot 2*bl+4, iters at 2*bl+5+j,
            # also overlapped by 2.
            LAGI = 2
            P1 = {}
            for s in range(LAGI * NB + 7):
                pull(40)
                if s % LAGI == 0 and s // LAGI < NB:
                    bl = s // LAGI
                    pull(10000)          # block bl's GEMM fully emitted
                    block_start(bl % 2, 1 - bl % 2, bl)
                    gen = gemm_gen(bl + 1) if bl + 1 < NB else None
                for bl in range(max(0, (s - K0) // LAGI),
                                min(NB, s // LAGI + 1)):
                    j = s - LAGI * bl
                    if 0 <= j < K0:
                        iter_ops(bl % 2, 1 - bl % 2, bl, g0_tiles[bl],
                                 whh0t_s, j, K0)
                        if bl == NB - 1 and j == 1:
                            # stale h0-sequence for the tail block's early
                            # l1 start (iter-1 writes ping-pong 0)
                            nc.vector.tensor_copy(
                                H0STALE[:], HF[bl % 2][:, 0, :, 1:LP])
                        if j == K0 - 1:
                            nc.vector.tensor_copy(
                                H0FIN[:], HF[bl % 2][:, K0 % 2, :, 1:LP])
                for bl in range(NB):
                    last = bl == NB - 1
                    i1 = 4 if last else 2 + bl % 2
                    o1 = 2 + (NB % 2) if last else 2 + (1 - bl % 2)
                    off = 1 if last else 3
                    if s == LAGI * bl + off:
                        block_start(i1, o1, bl)
                        P1[bl] = l1_xp(H0STALE if last else H0FIN, last)
                    j1 = s - (LAGI * bl + off + 1)
                    if 0 <= j1 < K1:
                        if bl == NB - 1 and j1 == 1:
                            # xp1 correction now that l0's final h0 is in
                            nc.vector.tensor_sub(DHX[:], H0FIN[:], H0STALE[:])
                            for g in range(4):
                                nc.tensor.matmul(P1[bl][:, g],
                                                 wih1t_s[:, g, :], DHX[:],
                                                 start=False, stop=True,
                                                 skip_group_check=True)
                        iter_ops(i1, o1, bl, P1[bl], whh1t_s, j1, K1)

            # ---- final fc (reuses a corner of the last gates PSUM tile) ----
            fcp = g0_tiles[NB - 1][0:BS, 0, 0, 0:1]
            nc.tensor.matmul(fcp, HF[4][:, K1 % 2, :, L:LP],
                             wfct_s[:],
                             start=True, stop=True, skip_group_check=True)
            nc.scalar.activation(y_sb[:], fcp, AF.Identity,
                                 bias=bfcb_s[:])
            nc.sync.dma_start(yd[:], y_sb[:])

    nc.compile()
    return nc


_PROG = None


def _get_program():
    global _PROG
    if _PROG is None:
        _PROG = build_program()
    return _PROG


def prep_inputs(x, Wih0, Whh0, bih0, bhh0, Wih1, Whh1, bih1, bhh1, Wfc, bfc):
    """Host-side layout prep -> per-core in_maps."""
    bf = ml_dtypes.bfloat16
    x = np.asarray(x, np.float32)

    # weights: [4H, K] -> [K(part), gate, unit]; g-gate x2 (sigma(2a) trick)
    def gate_T(Wmat):
        A = np.asarray(Wmat, np.float32).reshape(4, 128, -1)  # g, j, k
        A = A.transpose(2, 0, 1).copy()                       # k, g, j
        A[:, 2, :] *= 2.0
        return np.ascontiguousarray(A)

    wih0t = gate_T(Wih0).reshape(KT, 128, 4, 128).transpose(1, 0, 2, 3)
    wih0bf = np.ascontiguousarray(wih0t).astype(bf)           # [128,KT,4,128]
    whh0t = gate_T(Whh0).astype(bf)                           # [128,4,128]
    whh1t = gate_T(Whh1).astype(bf)
    wih1t = gate_T(Wih1).astype(bf)

    def bias_s(ba, bb):
        b = (np.asarray(ba) + np.asarray(bb)).astype(np.float32)
        b = b.reshape(4, 128).copy()
        b[2] *= 2.0
        return b[None].astype(bf)                             # [1,4,128]

    b0s = bias_s(bih0, bhh0)
    b1s = bias_s(bih1, bhh1)
    ones = np.ones((1, NTOK), bf)
    wfct = np.asarray(Wfc, np.float32).T.astype(bf)           # [128,1]
    bfcb = np.full((BS, 1), np.asarray(bfc, np.float32)[0], np.float32)
    common = dict(wih0bf=wih0bf,
                  whh0t=whh0t, whh1t=whh1t, wih1t=wih1t,
                  b0s=b0s, b1s=b1s, ones256=ones,
                  wfct=wfct, bfcb=bfcb)

    in_maps = []
    for c in range(NCORES):
        xc = x[c * BS:(c + 1) * BS]                           # [BS, T, D]
        xt = xc.transpose(2, 0, 1)                            # [D, BS, T]
        # [KT, 128, BS, NB, L] -> [NB, 128, KT, BS, L]
        xr = xt.reshape(KT, 128, BS, NB, L).transpose(3, 1, 0, 2, 4)
        in_maps.append({"xsb": np.ascontiguousarray(xr).astype(bf), **common})
    return in_maps


def run(inputs, **kw):
    nc = _get_program()
    in_maps = prep_inputs(**inputs)
    res = run_bass_kernel_spmd(nc, in_maps, core_ids=list(range(NCORES)), **kw)
    y = np.concatenate([res.results[c]["y"] for c in range(NCORES)], axis=0)
    return y.astype(np.float32), res


def kernel(**inputs):
    y, _ = run(inputs)
    return y


if __name__ == "__main__":
    import sys
    if "--sim" in sys.argv:
        import trails.perfetto as _tp
        if not hasattr(_tp.LazyPerfetto, "add_counter"):
            def _add_counter(self, proc, track, ts_, val):
                self.update_counter(proc, track, int(ts_), float(val),
                                    unit="ns")
            _tp.LazyPerfetto.add_counter = _add_counter
        for _m in ("enable_explicit_ordering", "reserve_process_order"):
            if not hasattr(_tp.LazyPerfetto, _m):
                setattr(_tp.LazyPerfetto, _m, lambda self, *a, **k: None)
        from concourse.timeline_sim import TimelineSim
        nc = _get_program()
        ts = TimelineSim(nc, trace="--trace" in sys.argv)
        dur = ts.simulate()
        print(f"TimelineSim predicted duration: {dur:.0f} ns")
        if ts.perfetto is not None:
            ts.perfetto.save("/root/problem/timeline.pftrace")
            print("wrote /root/problem/timeline.pftrace")


# revision 9
# speedup vs baseline: 1.7374x; 1.0395x over previous
"""Trainium2 Bass kernel for a 2-layer LSTM binary classifier.

Block-parallel Picard iteration: instead of a latency-bound serial
recurrence (~1.8us/step chain), process T in blocks of L=32 steps.
Within a block, iterate K times:
  gates = xp + Whh @ h_field        (h_field = stale estimates, bf16)
  sig   = sigmoid(gates)            (one big ACT op)
  u'    = (sig_g2 - 0.5) * sig_i    (c~ = c/2 space)
  c~    = scan: c~ = sig_f * c~ + u'   (exact, tensor_tensor_scan)
  tanh_c = tanh(2*c~)               (ACT, scale=2)
  h_new = sig_o * tanh_c            (bf16)
Per iteration the gates PSUM is updated with Whh @ (h_new - h_old), so
the matmuls stay small and the PSUM accumulates the converged value.
Convergence factor ~0.1/iter (measured): K0=K1=4 iterations give
final rel err ~2.2e-3 (incl bf16 h-fields), vs the 2e-2 gate.

The per-example scan boundary is handled with a gap slot: fields have
L+1 slots per example; slot 0 carries (f=0, u=c~_init) so one
tensor_tensor_scan over the whole [128, 8*(L+1)] field resets correctly
at each example boundary and performs the cross-block state handoff.

Sharding: data-parallel over batch (64 -> 8 cores x 8 examples), all
weights replicated.  The xp GEMM (all-bf16; one batched HWDGE DMA per
block) writes directly into each block's gates PSUM tile and is
emitted incrementally between iteration ops.  Both layers' blocks are
slot-scheduled as overlapping speculative strands (a block starts two
iteration-slots before its predecessor finishes, re-copying the
predecessor's latest h0/c~ handoff each iteration; iters 1+ see the
converged state), so the dependency wall shrinks toward engine
capacity.
"""

import numpy as np
import ml_dtypes

import concourse.bass as bass
import concourse.tile as tile
from concourse import bacc, mybir
from concourse.bass_utils import run_bass_kernel_spmd

F32 = mybir.dt.float32
F32R = mybir.dt.float32r
BF16 = mybir.dt.bfloat16
AF = mybir.ActivationFunctionType
OP = mybir.AluOpType

H = 128          # hidden
D = 2048         # input size
B = 64           # batch
T = 256          # seq len
NCORES = 8
BS = B // NCORES          # 8 examples per core
KT = D // 128             # 16 k-tiles of the input GEMM
L = 32                    # picard block length (timesteps)
NB = T // L               # 8 blocks
LP = L + 1                # field slots per example (slot 0 = gap/init)
K0 = 3                    # picard iterations, layer 0
K1 = 3                    # picard iterations, layer 1
K1L = 4                   # extra polish for the last l1 block (feeds fc)
NTOK = BS * L             # 256 tokens per block


def build_program():
    nc = bacc.Bacc("TRN2", target_bir_lowering=False, debug=False,
                   enable_asserts=False)

    # ---- DRAM I/O ----
    xsbd = nc.dram_tensor("xsb", [NB, 128, KT, BS, L], BF16,
                          kind="ExternalInput").ap()
    wih0bfd = nc.dram_tensor("wih0bf", [128, KT, 4, 128], BF16,
                             kind="ExternalInput").ap()
    whh0d = nc.dram_tensor("whh0t", [128, 4, 128], BF16,
                           kind="ExternalInput").ap()
    whh1d = nc.dram_tensor("whh1t", [128, 4, 128], BF16,
                           kind="ExternalInput").ap()
    wih1d = nc.dram_tensor("wih1t", [128, 4, 128], BF16,
                           kind="ExternalInput").ap()
    b0d = nc.dram_tensor("b0s", [1, 4, 128], BF16, kind="ExternalInput").ap()
    b1d = nc.dram_tensor("b1s", [1, 4, 128], BF16, kind="ExternalInput").ap()
    onesd = nc.dram_tensor("ones256", [1, NTOK], BF16,
                           kind="ExternalInput").ap()
    wfcd = nc.dram_tensor("wfct", [128, 1], BF16, kind="ExternalInput").ap()
    bfcd = nc.dram_tensor("bfcb", [BS, 1], F32, kind="ExternalInput").ap()
    yd = nc.dram_tensor("y", [BS, 1], F32, kind="ExternalOutput").ap()

    with tile.TileContext(nc) as tc, \
            tc.tile_pool(name="persist", bufs=1) as pp:
        # ---- persistent SBUF ----
        wih0bf_s = pp.tile([128, KT, 4, 128], BF16, name="wih0bf_s")
        whh0t_s = pp.tile([128, 4, 128], BF16, name="whh0t_s")
        whh1t_s = pp.tile([128, 4, 128], BF16, name="whh1t_s")
        wih1t_s = pp.tile([128, 4, 128], BF16, name="wih1t_s")
        b0s_s = pp.tile([1, 4, 128], BF16, name="b0s_s")
        b1s_s = pp.tile([1, 4, 128], BF16, name="b1s_s")
        ones_s = pp.tile([1, NTOK], BF16, name="ones_s")
        wfct_s = pp.tile([128, 1], BF16, name="wfct_s")
        bfcb_s = pp.tile([BS, 1], F32, name="bfcb_s")
        y_sb = pp.tile([BS, 1], F32, name="y_sb")

        # picard fields: each layer has two sets (overlapping blocks):
        # layer 0 -> idx bl%2, layer 1 -> idx 2 + bl%2
        SIG = [pp.tile([128, 4, BS, LP], F32, name=f"SIG{i}") for i in range(5)]
        U = [pp.tile([128, BS, LP], F32, name=f"U{i}") for i in range(5)]
        CT = [pp.tile([128, BS, LP], F32, name=f"CT{i}") for i in range(5)]
        TC = [pp.tile([128, BS, L], F32, name=f"TC{i}") for i in range(5)]
        HF = [pp.tile([128, 2, BS, LP], BF16, name=f"HF{i}") for i in range(5)]
        DH = [pp.tile([128, BS, L], BF16, name=f"DH{i}") for i in range(5)]
        H0FIN = pp.tile([128, BS, L], BF16, name="H0FIN")
        H0STALE = pp.tile([128, BS, L], BF16, name="H0STALE")
        DHX = pp.tile([128, BS, L], BF16, name="DHX")

        # GEMM bias consts must precede block-0 x DMAs on the sync queue
        nc.sync.dma_start(b0s_s[:], b0d[:])
        nc.sync.dma_start(ones_s[:], onesd[:])

        # one-time zero init: gap slots (f-gate slot0 must be exactly 0 so
        # the scan resets at example boundaries), block-0 state
        for l in range(5):
            nc.vector.memset(SIG[l][:, :, :, 0:1], 0.0)
            nc.vector.memset(U[l][:, :, 0:1], 0.0)
            nc.vector.memset(HF[l][:], 0.0)

        def _late_const_dmas():
            # on the gpsimd/SWDGE queue: Pool is otherwise idle, keeping
            # these small consts off the SP/ACT queues that carry the
            # prologue x/weight slabs (each dma_start costs ~0.6us of its
            # queue's sequencer + HWDGE slot)
            nc.gpsimd.dma_start(whh0t_s[:], whh0d[:])
            nc.gpsimd.dma_start(whh1t_s[:], whh1d[:])
            nc.gpsimd.dma_start(wih1t_s[:], wih1d[:])
            nc.gpsimd.dma_start(b1s_s[:], b1d[:])
            nc.gpsimd.dma_start(wfct_s[:], wfcd[:])
            nc.gpsimd.dma_start(bfcb_s[:], bfcd[:])

        with (
            tc.tile_pool(name="xchunk", bufs=2) as x_pool,
            tc.tile_pool(name="g0ps", bufs=2, space="PSUM") as g0_pool,
            tc.tile_pool(name="g1ps", bufs=2, space="PSUM") as g1_pool,
        ):
            g0_tiles = {}

            def gemm_gen(bl):
                """Emit block bl's xp GEMM directly into its gates PSUM."""
                xt = x_pool.tile([128, KT, BS, L], BF16, name="xt")
                if bl == 0:
                    # interleave x / weight quarter-slabs (x on the SP queue,
                    # weights on the ACT queue) so block-0 k-matmuls start as
                    # soon as their k-range has landed
                    for q in range(0, KT, 4):
                        nc.sync.dma_start(xt[:, q:q + 4], xsbd[0, :, q:q + 4])
                        nc.scalar.dma_start(wih0bf_s[:, q:q + 4],
                                            wih0bfd[:, q:q + 4])
                        if q == 0:
                            _late_const_dmas()
                        yield 1
                else:
                    nc.sync.dma_start(xt[:], xsbd[bl])
                    yield 1
                P = g0_pool.tile([128, 4, BS, L], F32, name="g0")
                g0_tiles[bl] = P
                # start=True clears the whole bank's has_written bits; the
                # tile spans 2 banks (2 gates each) so only g0/g2 may start
                for g in range(4):
                    nc.tensor.matmul(P[:, g], b0s_s[:, g, :], ones_s[:],
                                     start=(g % 2 == 0), stop=False,
                                     skip_group_check=True)
                    yield 1
                for k in range(KT):
                    for g in range(4):
                        nc.tensor.matmul(P[:, g], wih0bf_s[:, k, g, :],
                                         xt[:, k],
                                         start=False, stop=(k == KT - 1),
                                         skip_group_check=True)
                        yield 1

            gen = None

            def pull(n):
                nonlocal gen
                if gen is None:
                    return
                for _ in range(n):
                    if next(gen, None) is None:
                        gen = None
                        break

            def block_start(idx, other, bl):
                """Speculative handoff: block bl's init state comes from the
                sibling strand's latest iterate (final state is re-copied
                during iters 1+)."""
                nc.vector.memset(HF[idx][:, 0, :, 1:LP], 0.0)
                if bl > 0:
                    nc.vector.tensor_copy(HF[idx][:, 0, :, 0:1],
                                          HF[other][:, 0, :, L:LP])
                    nc.vector.tensor_copy(U[idx][:, :, 0:1],
                                          CT[other][:, :, L:LP])

            def iter_ops(idx, other, bl, P, whh_s, j, K, srcpp_ov=None):
                cur, nxt = j % 2, (j + 1) % 2
                # refresh the speculative h0 (slot 0) / c~ init from the
                # sibling strand's latest iterate; emitted ahead of the
                # chain (only WAR-ordered against iter j-1) so it runs
                # during the matmul/sigmoid wait
                if bl > 0:
                    srcpp = (srcpp_ov[j] if srcpp_ov is not None
                             else (min(j + 2, K - 1) + 1) % 2)
                    nc.vector.tensor_copy(HF[idx][:, nxt, :, 0:1],
                                          HF[other][:, srcpp, :, L:LP])
                    if j >= 1:
                        nc.vector.tensor_copy(U[idx][:, :, 0:1],
                                              CT[other][:, :, L:LP])
                rhs = HF[idx][:, 0, :, 0:L] if j == 0 else DH[idx][:]
                for g in range(4):
                    nc.tensor.matmul(P[:, g], whh_s[:, g, :], rhs,
                                     start=False, stop=True,
                                     skip_group_check=True)
                nc.scalar.activation(SIG[idx][:, 0:3, :, 1:LP], P[:, 0:3],
                                     AF.Sigmoid)
                nc.scalar.activation(SIG[idx][:, 3, :, 1:LP], P[:, 3],
                                     AF.Sigmoid)
                nc.vector.scalar_tensor_tensor(
                    U[idx][:, :, 1:LP], SIG[idx][:, 2, :, 1:LP], 0.5,
                    SIG[idx][:, 0, :, 1:LP],
                    op0=OP.subtract, op1=OP.mult)
                nc.vector.tensor_tensor_scan(
                    CT[idx][:].rearrange("p a b -> p (a b)"),
                    SIG[idx][:, 1].rearrange("p a b -> p (a b)"),
                    U[idx][:].rearrange("p a b -> p (a b)"),
                    0.0, op0=OP.mult, op1=OP.add)
                nc.scalar.activation(TC[idx][:], CT[idx][:, :, 1:LP],
                                     AF.Tanh, scale=2.0)
                nc.vector.tensor_mul(HF[idx][:, nxt, :, 1:LP],
                                     SIG[idx][:, 3, :, 1:LP], TC[idx][:])
                if j < K - 1:
                    nc.vector.tensor_sub(DH[idx][:], HF[idx][:, nxt, :, 0:L],
                                         HF[idx][:, cur, :, 0:L])

            def l1_xp(hsrc, from_g0=False):
                pool = g0_pool if from_g0 else g1_pool
                P1 = pool.tile([128, 4, BS, L], F32,
                               name="g0" if from_g0 else "g1")
                for g in range(4):
                    nc.tensor.matmul(P1[:, g], b1s_s[:, g, :], ones_s[:],
                                     start=(g % 2 == 0), stop=False,
                                     skip_group_check=True)
                for g in range(4):
                    nc.tensor.matmul(P1[:, g], wih1t_s[:, g, :], hsrc[:],
                                     start=False, stop=True,
                                     skip_group_check=True)
                return P1

            # ---- prologue: block-0 GEMM ----
            gen = gemm_gen(0)
            pull(10000)

            # ---- slot-scheduled wavefront ----
            # l0 block bl: iters j at slot 2*bl + j; blocks overlap by 2
            # iterations (a block's iters 1+ see its predecessor's final
            # state).  l1 block bl: xp at slot 2*bl+4, iters at 2*bl+5+j,
            # also overlapped by 2.
            LAGI = 2
            P1 = {}
            for s in range(LAGI * NB + 7):
                pull(40)
                if s % LAGI == 0 and s // LAGI < NB:
                    bl = s // LAGI
                    pull(10000)          # block bl's GEMM fully emitted
                    block_start(bl % 2, 1 - bl % 2, bl)
                    gen = gemm_gen(bl + 1) if bl + 1 < NB else None
                for bl in range(max(0, (s - K0) // LAGI),
                                min(NB, s // LAGI + 1)):
                    j = s - LAGI * bl
                    if 0 <= j < K0:
                        iter_ops(bl % 2, 1 - bl % 2, bl, g0_tiles[bl],
                                 whh0t_s, j, K0)
                        if bl == NB - 1 and j == 1:
                            # stale h0-sequence for the tail block's early
                            # l1 start (iter-1 writes ping-pong 0)
                            nc.vector.tensor_copy(
                                H0STALE[:], HF[bl % 2][:, 0, :, 1:LP])
                        if j == K0 - 1:
                            nc.vector.tensor_copy(
                                H0FIN[:], HF[bl % 2][:, K0 % 2, :, 1:LP])
                for bl in range(NB):
                    last = bl == NB - 1
                    i1 = 4 if last else 2 + bl % 2
                    o1 = 2 + (NB % 2) if last else 2 + (1 - bl % 2)
                    off = 1 if last else 3
                    if s == LAGI * bl + off:
                        block_start(i1, o1, bl)
                        P1[bl] = l1_xp(H0STALE if last else H0FIN, last)
                    j1 = s - (LAGI * bl + off + 1)
                    Kb = K1L if last else K1
                    if 0 <= j1 < Kb:
                        if last and j1 == 1:
                            # xp1 correction now that l0's final h0 is in
                            nc.vector.tensor_sub(DHX[:], H0FIN[:], H0STALE[:])
                            for g in range(4):
                                nc.tensor.matmul(P1[bl][:, g],
                                                 wih1t_s[:, g, :], DHX[:],
                                                 start=False, stop=True,
                                                 skip_group_check=True)
                        # last block's sibling has K1=3 slots: its latest
                        # written ping-pong by emission time is [1,0,1,1]
                        iter_ops(i1, o1, bl, P1[bl], whh1t_s, j1, Kb,
                                 srcpp_ov=([1, 0, 1, 1] if last else None))

            # ---- final fc (reuses a corner of the last gates PSUM tile) ----
            fcp = g0_tiles[NB - 1][0:BS, 0, 0, 0:1]
            nc.tensor.matmul(fcp, HF[4][:, K1L % 2, :, L:LP],
                             wfct_s[:],
                             start=True, stop=True, skip_group_check=True)
            nc.scalar.activation(y_sb[:], fcp, AF.Identity,
                                 bias=bfcb_s[:])
            nc.sync.dma_start(yd[:], y_sb[:])

    nc.compile()
    return nc


_PROG = None


def _get_program():
    global _PROG
    if _PROG is None:
        _PROG = build_program()
    return _PROG


def prep_inputs(x, Wih0, Whh0, bih0, bhh0, Wih1, Whh1, bih1, bhh1, Wfc, bfc):
    """Host-side layout prep -> per-core in_maps."""
    bf = ml_dtypes.bfloat16
    x = np.asarray(x, np.float32)

    # weights: [4H, K] -> [K(part), gate, unit]; g-gate x2 (sigma(2a) trick)
    def gate_T(Wmat):
        A = np.asarray(Wmat, np.float32).reshape(4, 128, -1)  # g, j, k
        A = A.transpose(2, 0, 1).copy()                       # k, g, j
        A[:, 2, :] *= 2.0
        return np.ascontiguousarray(A)

    wih0t = gate_T(Wih0).reshape(KT, 128, 4, 128).transpose(1, 0, 2, 3)
    wih0bf = np.ascontiguousarray(wih0t).astype(bf)           # [128,KT,4,128]
    whh0t = gate_T(Whh0).astype(bf)                           # [128,4,128]
    whh1t = gate_T(Whh1).astype(bf)
    wih1t = gate_T(Wih1).astype(bf)

    def bias_s(ba, bb):
        b = (np.asarray(ba) + np.asarray(bb)).astype(np.float32)
        b = b.reshape(4, 128).copy()
        b[2] *= 2.0
        return b[None].astype(bf)                             # [1,4,128]

    b0s = bias_s(bih0, bhh0)
    b1s = bias_s(bih1, bhh1)
    ones = np.ones((1, NTOK), bf)
    wfct = np.asarray(Wfc, np.float32).T.astype(bf)           # [128,1]
    bfcb = np.full((BS, 1), np.asarray(bfc, np.float32)[0], np.float32)
    common = dict(wih0bf=wih0bf,
                  whh0t=whh0t, whh1t=whh1t, wih1t=wih1t,
                  b0s=b0s, b1s=b1s, ones256=ones,
                  wfct=wfct, bfcb=bfcb)

    in_maps = []
    for c in range(NCORES):
        xc = x[c * BS:(c + 1) * BS]                           # [BS, T, D]
        xt = xc.transpose(2, 0, 1)                            # [D, BS, T]
        # [KT, 128, BS, NB, L] -> [NB, 128, KT, BS, L]
        xr = xt.reshape(KT, 128, BS, NB, L).transpose(3, 1, 0, 2, 4)
        in_maps.append({"xsb": np.ascontiguousarray(xr).astype(bf), **common})
    return in_maps


def run(inputs, **kw):
    nc = _get_program()
    in_maps = prep_inputs(**inputs)
    res = run_bass_kernel_spmd(nc, in_maps, core_ids=list(range(NCORES)), **kw)
    y = np.concatenate([res.results[c]["y"] for c in range(NCORES)], axis=0)
    return y.astype(np.float32), res


def kernel(**inputs):
    y, _ = run(inputs)
    return y


if __name__ == "__main__":
    import sys
    if "--sim" in sys.argv:
        import trails.perfetto as _tp
        if not hasattr(_tp.LazyPerfetto, "add_counter"):
            def _add_counter(self, proc, track, ts_, val):
                self.update_counter(proc, track, int(ts_), float(val),
                                    unit="ns")
            _tp.LazyPerfetto.add_counter = _add_counter
        for _m in ("enable_explicit_ordering", "reserve_process_order"):
            if not hasattr(_tp.LazyPerfetto, _m):
                setattr(_tp.LazyPerfetto, _m, lambda self, *a, **k: None)
        from concourse.timeline_sim import TimelineSim
        nc = _get_program()
        ts = TimelineSim(nc, trace="--trace" in sys.argv)
        dur = ts.simulate()
        print(f"TimelineSim predicted duration: {dur:.0f} ns")
        if ts.perfetto is not None:
            ts.perfetto.save("/root/problem/timeline.pftrace")
            print("wrote /root/problem/timeline.pftrace")
